# revision 5
# baseline (speedup 1.0000x reference)
"""DCRNN (2x GCNConv + GRU-over-nodes + Linear) on 8 Trainium2 cores.

Strategy
--------
* GCN layers: the normalized adjacency (A+I with D^-1/2 scaling) is built
  densely on the host in fp16 and sharded row-wise across the 8 cores
  (1250 rows/core + a 64-row left halo for the GRU).  Each GCN layer is a
  dense [rows, 10000] @ [10000, 256] fp16 matmul on the TensorEngine, with
  everything kept transposed ([feat, node] layout) so no on-device
  transposes are needed.  One AllGather shares h1 between cores.
* GRU over the 10000-node sequence is strictly sequential in the
  reference.  We solve it with K fixed-point sweeps: gates (r,z,n) are
  evaluated from the previous sweep's hidden state (one big matmul +
  pointwise), then h_t = z_t*h_{t-1} + (1-z_t)*n_t is applied EXACTLY with
  the DVE affine-scan primitive (tensor_tensor_scan).  The per-step decay
  |dh_t/dh_{t-1}| ~ 0.74 makes this converge geometrically; the 64-row
  halo makes the cores fully independent (boundary error ~ 0.74^64).
* Final Linear runs on the node shard; host concatenates the 8 shards.
"""

import numpy as np

NUM_NODES = 10000
IN_FEAT = 64
HID = 256
OUT = 3
CORES = 8
ROWS = NUM_NODES // CORES          # 1250
HALO = 64
L = ROWS + HALO                    # 1314 local sequence length
SWEEPS = 14
MT = 79                            # ceil(10000/128) K-tiles (79*128 = 10112 > 10000)
KP = 128

_CACHE = {}


def _chunks(total, step=512):
    return [(c, min(c + step, total)) for c in range(0, total, step)]


def build_program():
    import concourse.bass as bass
    import concourse.mybir as mybir
    import concourse.tile as tile
    from concourse import bacc

    f16 = mybir.dt.float16
    f32 = mybir.dt.float32
    AF = mybir.ActivationFunctionType
    ALU = mybir.AluOpType

    nc = bacc.Bacc("TRN2", num_devices=CORES)

    # K-padded node count so the K-tile loop is uniform (pad rows are zero).
    NPAD = MT * KP

    # ---- inputs ----
    a2t_d = nc.dram_tensor("a2t", [NPAD, L], f16, kind="ExternalInput")
    xt_d = nc.dram_tensor("xt", [IN_FEAT, NPAD], f16, kind="ExternalInput")
    w1_d = nc.dram_tensor("w1", [IN_FEAT, HID], f16, kind="ExternalInput")
    w2_d = nc.dram_tensor("w2", [HID, HID], f16, kind="ExternalInput")
    wiht_d = nc.dram_tensor("wiht", [HID, 3 * HID], f16, kind="ExternalInput")
    whht_d = nc.dram_tensor("whht", [HID, 3 * HID], f16, kind="ExternalInput")
    fcwt_d = nc.dram_tensor("fcwt", [HID, OUT], f16, kind="ExternalInput")
    ident_d = nc.dram_tensor("ident", [KP, KP], f16, kind="ExternalInput")
    b1c_d = nc.dram_tensor("b1c", [KP, 2], f32, kind="ExternalInput")
    b2c_d = nc.dram_tensor("b2c", [KP, 2], f32, kind="ExternalInput")
    gib_d = nc.dram_tensor("gib", [KP, 6], f32, kind="ExternalInput")
    bhn_d = nc.dram_tensor("bhn", [KP, 2], f32, kind="ExternalInput")
    fcb_d = nc.dram_tensor("fcb", [KP, 1], f32, kind="ExternalInput")
    patch_d = nc.dram_tensor("patch", [KP, 12], f32, kind="ExternalInput")
    out_d = nc.dram_tensor("out_t", [OUT, ROWS], f32, kind="ExternalOutput")

    with tile.TileContext(nc) as tc:
        with (
            tc.tile_pool(name="const", bufs=1) as cpool,
            tc.tile_pool(name="big", bufs=1) as big,
            tc.tile_pool(name="astream", bufs=3) as apool,
            tc.tile_pool(name="tmp", bufs=4) as tpool,
            tc.tile_pool(name="ps", bufs=1, space="PSUM") as pspool,
            tc.tile_pool(name="psxw", bufs=2, space="PSUM") as psxw,
            tc.tile_pool(name="dram", bufs=1, space="DRAM") as dpool,
        ):
            # ---- load constants ----
            xt_sb = cpool.tile([IN_FEAT, NPAD], f16)
            w1_sb = cpool.tile([IN_FEAT, HID], f16)
            w2_sb = cpool.tile([KP, 2, HID], f16)
            wiht_sb = cpool.tile([KP, 2, 3 * HID], f16)
            whht_sb = cpool.tile([KP, 2, 3 * HID], f16)
            fcwt_sb = cpool.tile([KP, 2, OUT], f16)
            ident_sb = cpool.tile([KP, KP], f16)
            b1c_sb = cpool.tile([KP, 2], f32)
            b2c_sb = cpool.tile([KP, 2], f32)
            gib_sb = cpool.tile([KP, 6], f32)
            bhn_sb = cpool.tile([KP, 2], f32)
            fcb_sb = cpool.tile([KP, 1], f32)
            patch_sb = cpool.tile([KP, 12], f32)

            nc.sync.dma_start(xt_sb[:], xt_d[:])
            nc.sync.dma_start(w1_sb[:], w1_d[:])
            for k in range(2):
                nc.sync.dma_start(w2_sb[:, k, :], w2_d[k * KP:(k + 1) * KP, :])
                nc.sync.dma_start(wiht_sb[:, k, :], wiht_d[k * KP:(k + 1) * KP, :])
                nc.sync.dma_start(whht_sb[:, k, :], whht_d[k * KP:(k + 1) * KP, :])
                nc.sync.dma_start(fcwt_sb[:, k, :], fcwt_d[k * KP:(k + 1) * KP, :])
            nc.sync.dma_start(ident_sb[:], ident_d[:])
            nc.sync.dma_start(b1c_sb[:], b1c_d[:])
            nc.sync.dma_start(b2c_sb[:], b2c_d[:])
            nc.sync.dma_start(gib_sb[:], gib_d[:])
            nc.sync.dma_start(bhn_sb[:], bhn_d[:])
            nc.sync.dma_start(fcb_sb[:], fcb_d[:])
            nc.sync.dma_start(patch_sb[:], patch_d[:])

            # ---- XW1 = x @ W1 in natural layout ([node(K), 256]) ----
            xw_sb = big.tile([KP, MT, HID], f16, tag="xw")
            for m in range(MT):
                ps = psxw.tile([KP, 512], f32, tag="xwps")
                nc.tensor.matmul(ps[:, :HID], xt_sb[:, m * KP:(m + 1) * KP],
                                 w1_sb[:], start=True, stop=True)
                if m % 2 == 0:
                    nc.scalar.activation(xw_sb[:, m, :], ps[:, :HID], AF.Copy)
                else:
                    nc.vector.tensor_copy(xw_sb[:, m, :], ps[:, :HID])

            # ---- GCN layer 1: h1T_loc = relu(bias + XW1.T @ A1) ----
            ch1 = _chunks(ROWS)
            ps1 = [[pspool.tile([KP, 512], f32, tag=f"g{mm * 3 + ci}", name=f"ps1_{mm}_{ci}")
                    for ci in range(3)] for mm in range(2)]
            for k in range(MT):
                at = apool.tile([KP, L], f16, tag="a")
                nc.sync.dma_start(at[:, :ROWS],
                                  a2t_d[k * KP:(k + 1) * KP, HALO:L])
                for mm in range(2):
                    lhsT = xw_sb[:, k, mm * KP:(mm + 1) * KP]
                    for ci, (c0, c1) in enumerate(ch1):
                        nc.tensor.matmul(ps1[mm][ci][:, :c1 - c0], lhsT,
                                         at[:, c0:c1],
                                         start=(k == 0), stop=(k == MT - 1))
            h1t_sb = big.tile([KP, 2, ROWS], f16)
            for mm in range(2):
                for ci, (c0, c1) in enumerate(ch1):
                    nc.scalar.activation(h1t_sb[:, mm, c0:c1],
                                         ps1[mm][ci][:, :c1 - c0], AF.Relu,
                                         bias=b1c_sb[:, mm:mm + 1])

            # ---- AllGather h1 across the 8 cores ----
            bounce_in = dpool.tile([2 * KP, ROWS], f16)
            gath = dpool.tile([CORES * 2 * KP, ROWS], f16, addr_space="Shared")
            for mm in range(2):
                nc.sync.dma_start(bounce_in[mm * KP:(mm + 1) * KP, :],
                                  h1t_sb[:, mm, :])
            nc.gpsimd.collective_compute(
                "AllGather",
                mybir.AluOpType.bypass,
                replica_groups=[list(range(CORES))],
                ins=[bounce_in.opt()],
                outs=[gath.opt()],
            )
            h1f_sb = big.tile([KP, 2, NPAD], f16)
            for mm in range(2):
                # zero K-padding tail once (columns NUM_NODES..NPAD)
                nc.vector.memset(h1f_sb[:, mm, NUM_NODES:], 0.0)
            for c in range(CORES):
                for mm in range(2):
                    nc.sync.dma_start(
                        h1f_sb[:, mm, c * ROWS:(c + 1) * ROWS],
                        gath[c * 2 * KP + mm * KP: c * 2 * KP + (mm + 1) * KP, :])

            # ---- XW2 = h1 @ W2 in natural layout ----
            xw2_sb = big.tile([KP, MT, HID], f16, tag="xw")
            for m in range(MT):
                ps = psxw.tile([KP, 512], f32, tag="xwps")
                for k in range(2):
                    nc.tensor.matmul(ps[:, :HID],
                                     h1f_sb[:, k, m * KP:(m + 1) * KP],
                                     w2_sb[:, k, :],
                                     start=(k == 0), stop=(k == 1))
                if m % 2 == 0:
                    nc.scalar.activation(xw2_sb[:, m, :], ps[:, :HID], AF.Copy)
                else:
                    nc.vector.tensor_copy(xw2_sb[:, m, :], ps[:, :HID])

            # ---- GCN layer 2 over the extended (halo) shard ----
            ch2 = _chunks(L)
            ps2 = [[pspool.tile([KP, 512], f32, tag=f"g{mm * 3 + ci}", name=f"ps2_{mm}_{ci}")
                    for ci in range(3)] for mm in range(2)]
            for k in range(MT):
                at = apool.tile([KP, L], f16, tag="a")
                nc.sync.dma_start(at[:], a2t_d[k * KP:(k + 1) * KP, :])
                for mm in range(2):
                    lhsT = xw2_sb[:, k, mm * KP:(mm + 1) * KP]
                    for ci, (c0, c1) in enumerate(ch2):
                        nc.tensor.matmul(ps2[mm][ci][:, :c1 - c0], lhsT,
                                         at[:, c0:c1],
                                         start=(k == 0), stop=(k == MT - 1))
            h2t_sb = big.tile([KP, 2, L], f16)
            for mm in range(2):
                for ci, (c0, c1) in enumerate(ch2):
                    nc.scalar.activation(h2t_sb[:, mm, c0:c1],
                                         ps2[mm][ci][:, :c1 - c0], AF.Relu,
                                         bias=b2c_sb[:, mm:mm + 1])

            # ---- GI = W_ih @ h2T + (b_ih [+ b_hh for r,z]) ----
            gi_sb = big.tile([KP, 6, L], f16)
            for c0, c1 in ch2:
                psg = [pspool.tile([KP, 512], f32, tag=f"g{m}", name=f"psgi_{m}") for m in range(6)]
                for m in range(6):
                    for k in range(2):
                        nc.tensor.matmul(psg[m][:, :c1 - c0],
                                         wiht_sb[:, k, m * KP:(m + 1) * KP],
                                         h2t_sb[:, k, c0:c1],
                                         start=(k == 0), stop=(k == 1))
                    nc.scalar.activation(gi_sb[:, m, c0:c1], psg[m][:, :c1 - c0],
                                         AF.Identity, bias=gib_sb[:, m:m + 1])
            # per-core GI patch on the first HALO columns (core 0 kills its pads)
            for m in range(6):
                nc.vector.tensor_scalar(gi_sb[:, m, :HALO], gi_sb[:, m, :HALO],
                                        patch_sb[:, m:m + 1],
                                        patch_sb[:, 6 + m:7 + m],
                                        ALU.mult, ALU.add)

            # ---- GRU fixed-point sweeps ----
            hsh_sb = big.tile([KP, 2, L + 1], f16)
            for mm in range(2):
                nc.vector.memset(hsh_sb[:, mm, :], 0.0)
            for s in range(SWEEPS):
                z_sb = big.tile([KP, 2, L], f16, tag="Z")
                b_sb = big.tile([KP, 2, L], f16, tag="B")
                for c0, c1 in ch2:
                    cw = c1 - c0
                    psg = [pspool.tile([KP, 512], f32, tag=f"g{m}", name=f"psu_{m}")
                           for m in range(6)]
                    # u_rz = GI_rz (identity matmul) + W_hh_rz @ h_prev
                    for m in range(4):
                        nc.tensor.matmul(psg[m][:, :cw], ident_sb[:],
                                         gi_sb[:, m, c0:c1],
                                         start=True, stop=False)
                    for m in range(6):
                        for k in range(2):
                            nc.tensor.matmul(psg[m][:, :cw],
                                             whht_sb[:, k, m * KP:(m + 1) * KP],
                                             hsh_sb[:, k, c0:c1],
                                             start=(m >= 4 and k == 0),
                                             stop=(k == 1))
                    for mm in range(2):
                        r_t = tpool.tile([KP, 512], f16, tag="r")
                        zc_t = tpool.tile([KP, 512], f16, tag="zc")
                        ghn_t = tpool.tile([KP, 512], f16, tag="ghn")
                        t_t = tpool.tile([KP, 512], f16, tag="t")
                        un_t = tpool.tile([KP, 512], f16, tag="un")
                        n_t = tpool.tile([KP, 512], f16, tag="n")
                        nc.scalar.activation(r_t[:, :cw], psg[mm][:, :cw],
                                             AF.Sigmoid)
                        nc.scalar.activation(z_sb[:, mm, c0:c1],
                                             psg[2 + mm][:, :cw], AF.Sigmoid)
                        nc.scalar.activation(zc_t[:, :cw], psg[2 + mm][:, :cw],
                                             AF.Sigmoid, scale=-1.0)
                        nc.scalar.activation(ghn_t[:, :cw], psg[4 + mm][:, :cw],
                                             AF.Identity,
                                             bias=bhn_sb[:, mm:mm + 1])
                        nc.vector.tensor_mul(t_t[:, :cw], r_t[:, :cw],
                                             ghn_t[:, :cw])
                        nc.vector.tensor_add(un_t[:, :cw], t_t[:, :cw],
                                             gi_sb[:, 4 + mm, c0:c1])
                        nc.scalar.activation(n_t[:, :cw], un_t[:, :cw], AF.Tanh)
                        nc.vector.tensor_mul(b_sb[:, mm, c0:c1], zc_t[:, :cw],
                                             n_t[:, :cw])
                # exact h recurrence: h_t = z_t * h_{t-1} + (1-z_t) n_t
                for mm in range(2):
                    nc.vector.tensor_tensor_scan(
                        hsh_sb[:, mm, 1:L + 1], z_sb[:, mm, :], b_sb[:, mm, :],
                        0.0, ALU.mult, ALU.add)

            # ---- final Linear on the real rows (skip halo) ----
            out_sb = cpool.tile([4, ROWS], f32)
            for c0, c1 in ch1:
                cw = c1 - c0
                psf = psxw.tile([KP, 512], f32, tag="xwps")
                for k in range(2):
                    nc.tensor.matmul(psf[:OUT, :cw], fcwt_sb[:, k, :],
                                     hsh_sb[:, k, HALO + 1 + c0:HALO + 1 + c1],
                                     start=(k == 0), stop=(k == 1))
                nc.scalar.activation(out_sb[:OUT, c0:c1], psf[:OUT, :cw],
                                     AF.Identity, bias=fcb_sb[:OUT, :])
            nc.sync.dma_start(out_d[:], out_sb[:OUT, :])

    nc.compile()
    return nc


def host_prepare(inputs):
    """Build the per-core input maps from the full problem inputs."""
    x = np.asarray(inputs["x"], np.float32)
    ei = np.asarray(inputs["edge_index"])
    W1 = np.asarray(inputs["W1"], np.float32)
    b1 = np.asarray(inputs["b1"], np.float32)
    W2 = np.asarray(inputs["W2"], np.float32)
    b2 = np.asarray(inputs["b2"], np.float32)
    W_ih = np.asarray(inputs["W_ih"], np.float32)
    W_hh = np.asarray(inputs["W_hh"], np.float32)
    b_ih = np.asarray(inputs["b_ih"], np.float32)
    b_hh = np.asarray(inputs["b_hh"], np.float32)
    fc_w = np.asarray(inputs["fc_w"], np.float32)
    fc_b = np.asarray(inputs["fc_b"], np.float32)

    N = NUM_NODES
    NPAD = MT * KP
    src, dst = ei[0].astype(np.int64), ei[1].astype(np.int64)
    deg = np.bincount(dst, minlength=N).astype(np.float64) + 1.0
    dinv = 1.0 / np.sqrt(deg)
    # A_T[s, d] = normalization weight of edge s->d (plus self loops)
    at = np.zeros((N, N), np.float32)
    np.add.at(at, (src, dst), (dinv[src] * dinv[dst]).astype(np.float32))
    idx = np.arange(N)
    at[idx, idx] += (dinv * dinv).astype(np.float32)
    at16 = at.astype(np.float16)
    del at

    atp = np.zeros((NPAD, L), np.float16)

    common = {
        "xt": np.concatenate(
            [x.T.astype(np.float16),
             np.zeros((IN_FEAT, NPAD - N), np.float16)], axis=1),
        "w1": W1.astype(np.float16),
        "w2": W2.astype(np.float16),
        "wiht": W_ih.T.astype(np.float16),
        "whht": W_hh.T.astype(np.float16),
        "fcwt": fc_w.T.astype(np.float16),
        "ident": np.eye(KP, dtype=np.float16),
        "b1c": b1.reshape(2, KP).T.astype(np.float32).copy(),
        "b2c": b2.reshape(2, KP).T.astype(np.float32).copy(),
        "gib": (b_ih + np.concatenate([b_hh[:2 * HID],
                                       np.zeros(HID, np.float32)])
                ).reshape(6, KP).T.astype(np.float32).copy(),
        "bhn": b_hh[2 * HID:].reshape(2, KP).T.astype(np.float32).copy(),
        "fcb": np.concatenate([fc_b, np.zeros(KP - OUT, np.float32)]
                              ).reshape(KP, 1),
    }

    in_maps = []
    for c in range(CORES):
        r0, r1 = c * ROWS, (c + 1) * ROWS
        a2t = atp.copy()
        if c == 0:
            a2t[:N, HALO:] = at16[:, r0:r1]
        else:
            a2t[:N, :] = at16[:, r0 - HALO:r1]
        patch = np.zeros((KP, 12), np.float32)
        if c == 0:
            # mul=0; add=-60 for r,z gate tiles, 0 for n tiles -> pad cols
            # produce exactly h=0 so row 0 starts from the true h0=0.
            patch[:, 6:10] = -60.0
        else:
            patch[:, 0:6] = 1.0
        in_maps.append({**common, "a2t": a2t, "patch": patch})
    return in_maps


def assemble_output(results):
    outs = [r["out_t"].T for r in results]          # each [ROWS, OUT]
    full = np.concatenate(outs, axis=0).astype(np.float32)
    return full[None]                               # [1, N, OUT]


def kernel(**inputs) -> np.ndarray:
    from concourse import bass_utils

    if "nc" not in _CACHE:
        _CACHE["nc"] = build_program()
    nc = _CACHE["nc"]
    in_maps = host_prepare(inputs)
    res = bass_utils.run_bass_kernel_spmd(
        nc, in_maps, core_ids=list(range(CORES)))
    return assemble_output(res.results)


if __name__ == "__main__":
    import reference

    inputs = {k: np.asarray(v) for k, v in reference.setup_inputs().items()}
    out = kernel(**inputs)
    print("kernel out", out.shape, out.dtype)
    np.save("/root/problem/kernel_out.npy", out)


# revision 7
# speedup vs baseline: 1.1834x; 1.1834x over previous
"""DCRNN (2x GCNConv + GRU-over-nodes + Linear) on 8 Trainium2 cores.

Strategy
--------
* GCN layers: the normalized adjacency (A+I with D^-1/2 scaling) is built
  densely on the host in fp16 and sharded row-wise across the 8 cores
  (1250 rows/core + a 64-row left halo for the GRU).  Each GCN layer is a
  dense [rows, 10000] @ [10000, 256] fp16 matmul on the TensorEngine, with
  everything kept transposed ([feat, node] layout) so no on-device
  transposes are needed.  One AllGather shares h1 between cores.
* GRU over the 10000-node sequence is strictly sequential in the
  reference.  We solve it with K fixed-point sweeps: gates (r,z,n) are
  evaluated from the previous sweep's hidden state (one big matmul +
  pointwise), then h_t = z_t*h_{t-1} + (1-z_t)*n_t is applied EXACTLY with
  the DVE affine-scan primitive (tensor_tensor_scan).  The per-step decay
  |dh_t/dh_{t-1}| ~ 0.74 makes this converge geometrically; the 64-row
  halo makes the cores fully independent (boundary error ~ 0.74^64).
* Final Linear runs on the node shard; host concatenates the 8 shards.
"""

import numpy as np

NUM_NODES = 10000
IN_FEAT = 64
HID = 256
OUT = 3
CORES = 8
ROWS = NUM_NODES // CORES          # 1250
HALO = 64
L = ROWS + HALO                    # 1314 local sequence length
SWEEPS = 12
MT = 79                            # ceil(10000/128) K-tiles (79*128 = 10112 > 10000)
KP = 128

_CACHE = {}


def _chunks(total, step=512):
    return [(c, min(c + step, total)) for c in range(0, total, step)]


def build_program():
    import concourse.bass as bass
    import concourse.mybir as mybir
    import concourse.tile as tile
    from concourse import bacc

    f16 = mybir.dt.float16
    f32 = mybir.dt.float32
    AF = mybir.ActivationFunctionType
    ALU = mybir.AluOpType

    nc = bacc.Bacc("TRN2", num_devices=CORES)

    # K-padded node count so the K-tile loop is uniform (pad rows are zero).
    NPAD = MT * KP

    # ---- inputs ----
    a2t_d = nc.dram_tensor("a2t", [NPAD, L], f16, kind="ExternalInput")
    xt_d = nc.dram_tensor("xt", [IN_FEAT, NPAD], f16, kind="ExternalInput")
    w1_d = nc.dram_tensor("w1", [IN_FEAT, HID], f16, kind="ExternalInput")
    w2_d = nc.dram_tensor("w2", [HID, HID], f16, kind="ExternalInput")
    wiht_d = nc.dram_tensor("wiht", [HID, 3 * HID], f16, kind="ExternalInput")
    whht_d = nc.dram_tensor("whht", [HID, 3 * HID], f16, kind="ExternalInput")
    fcwt_d = nc.dram_tensor("fcwt", [HID, OUT], f16, kind="ExternalInput")
    ident_d = nc.dram_tensor("ident", [KP, KP], f16, kind="ExternalInput")
    b1c_d = nc.dram_tensor("b1c", [KP, 2], f32, kind="ExternalInput")
    b2c_d = nc.dram_tensor("b2c", [KP, 2], f32, kind="ExternalInput")
    gib_d = nc.dram_tensor("gib", [KP, 6], f32, kind="ExternalInput")
    bhn_d = nc.dram_tensor("bhn", [KP, 2], f32, kind="ExternalInput")
    fcb_d = nc.dram_tensor("fcb", [KP, 1], f32, kind="ExternalInput")
    patch_d = nc.dram_tensor("patch", [KP, 12], f32, kind="ExternalInput")
    out_d = nc.dram_tensor("out_t", [OUT, ROWS], f32, kind="ExternalOutput")

    with tile.TileContext(nc) as tc:
        with (
            tc.tile_pool(name="const", bufs=1) as cpool,
            tc.tile_pool(name="big", bufs=1) as big,
            tc.tile_pool(name="astream", bufs=6) as apool,
            tc.tile_pool(name="tmp", bufs=4) as tpool,
            tc.tile_pool(name="ps", bufs=1, space="PSUM") as pspool,
            tc.tile_pool(name="psxw", bufs=2, space="PSUM") as psxw,
            tc.tile_pool(name="dram", bufs=1, space="DRAM") as dpool,
        ):
            # ---- load constants ----
            xt_sb = cpool.tile([IN_FEAT, NPAD], f16)
            w1_sb = cpool.tile([IN_FEAT, HID], f16)
            w2_sb = cpool.tile([KP, 2, HID], f16)
            wiht_sb = cpool.tile([KP, 2, 3 * HID], f16)
            whht_sb = cpool.tile([KP, 2, 3 * HID], f16)
            fcwt_sb = cpool.tile([KP, 2, OUT], f16)
            ident_sb = cpool.tile([KP, KP], f16)
            b1c_sb = cpool.tile([KP, 2], f32)
            b2c_sb = cpool.tile([KP, 2], f32)
            gib_sb = cpool.tile([KP, 6], f32)
            bhn_sb = cpool.tile([KP, 2], f32)
            fcb_sb = cpool.tile([KP, 1], f32)
            patch_sb = cpool.tile([KP, 12], f32)

            nc.sync.dma_start(xt_sb[:], xt_d[:])
            nc.sync.dma_start(w1_sb[:], w1_d[:])
            for k in range(2):
                nc.sync.dma_start(w2_sb[:, k, :], w2_d[k * KP:(k + 1) * KP, :])
                nc.sync.dma_start(wiht_sb[:, k, :], wiht_d[k * KP:(k + 1) * KP, :])
                nc.sync.dma_start(whht_sb[:, k, :], whht_d[k * KP:(k + 1) * KP, :])
                nc.sync.dma_start(fcwt_sb[:, k, :], fcwt_d[k * KP:(k + 1) * KP, :])
            nc.sync.dma_start(ident_sb[:], ident_d[:])
            nc.sync.dma_start(b1c_sb[:], b1c_d[:])
            nc.sync.dma_start(b2c_sb[:], b2c_d[:])
            nc.sync.dma_start(gib_sb[:], gib_d[:])
            nc.sync.dma_start(bhn_sb[:], bhn_d[:])
            nc.sync.dma_start(fcb_sb[:], fcb_d[:])
            nc.sync.dma_start(patch_sb[:], patch_d[:])

            # ---- XW1 = x @ W1 in natural layout ([node(K), 256]) ----
            xw_sb = big.tile([KP, MT, HID], f16, tag="xw")
            for m in range(MT):
                ps = psxw.tile([KP, 512], f32, tag="xwps")
                nc.tensor.matmul(ps[:, :HID], xt_sb[:, m * KP:(m + 1) * KP],
                                 w1_sb[:], start=True, stop=True)
                if m % 2 == 0:
                    nc.scalar.activation(xw_sb[:, m, :], ps[:, :HID], AF.Copy)
                else:
                    nc.vector.tensor_copy(xw_sb[:, m, :], ps[:, :HID])

            # ---- GCN layer 1: h1T_loc = relu(bias + XW1.T @ A1) ----
            ch1 = _chunks(ROWS)
            ps1 = [[pspool.tile([KP, 512], f32, tag=f"g{mm * 3 + ci}", name=f"ps1_{mm}_{ci}")
                    for ci in range(3)] for mm in range(2)]
            for k in range(MT):
                at = apool.tile([KP, L], f16, tag="a")
                nc.sync.dma_start(at[:, :ROWS],
                                  a2t_d[k * KP:(k + 1) * KP, HALO:L])
                for mm in range(2):
                    lhsT = xw_sb[:, k, mm * KP:(mm + 1) * KP]
                    for ci, (c0, c1) in enumerate(ch1):
                        nc.tensor.matmul(ps1[mm][ci][:, :c1 - c0], lhsT,
                                         at[:, c0:c1],
                                         start=(k == 0), stop=(k == MT - 1))
            h1t_sb = big.tile([KP, 2, ROWS], f16)
            for mm in range(2):
                for ci, (c0, c1) in enumerate(ch1):
                    nc.scalar.activation(h1t_sb[:, mm, c0:c1],
                                         ps1[mm][ci][:, :c1 - c0], AF.Relu,
                                         bias=b1c_sb[:, mm:mm + 1])

            # ---- XW2 shard = h1_loc @ W2 (natural layout), then AllGather XW2 ----
            nloc = (ROWS + KP - 1) // KP          # 10 M-tiles (last is 98 rows)
            xw2l_sb = cpool.tile([KP, nloc, HID], f16)
            for j in range(nloc):
                rw = min(KP, ROWS - j * KP)
                ps = psxw.tile([KP, 512], f32, tag="xwps")
                for k in range(2):
                    nc.tensor.matmul(ps[:rw, :HID],
                                     h1t_sb[:, k, j * KP:j * KP + rw],
                                     w2_sb[:, k, :],
                                     start=(k == 0), stop=(k == 1))
                if j % 2 == 0:
                    nc.scalar.activation(xw2l_sb[:rw, j, :], ps[:rw, :HID], AF.Copy)
                else:
                    nc.vector.tensor_copy(xw2l_sb[:rw, j, :], ps[:rw, :HID])
            bounce_in = dpool.tile([ROWS, HID], f16)
            gath = dpool.tile([NUM_NODES, HID], f16, addr_space="Shared")
            for j in range(nloc):
                rw = min(KP, ROWS - j * KP)
                nc.sync.dma_start(bounce_in[j * KP:j * KP + rw, :],
                                  xw2l_sb[:rw, j, :])
            nc.gpsimd.collective_compute(
                "AllGather",
                mybir.AluOpType.bypass,
                replica_groups=[list(range(CORES))],
                ins=[bounce_in.opt()],
                outs=[gath.opt()],
            )
            xw2_sb = big.tile([KP, MT, HID], f16, tag="xw")
            nc.vector.memset(xw2_sb[:, MT - 1, :], 0.0)
            for k in range(MT):
                rw = min(KP, NUM_NODES - k * KP)
                nc.sync.dma_start(xw2_sb[:rw, k, :],
                                  gath[k * KP:k * KP + rw, :])

            # ---- GCN layer 2 over the extended (halo) shard ----
            ch2 = _chunks(L)
            ps2 = [[pspool.tile([KP, 512], f32, tag=f"g{mm * 3 + ci}", name=f"ps2_{mm}_{ci}")
                    for ci in range(3)] for mm in range(2)]
            for k in range(MT):
                at = apool.tile([KP, L], f16, tag="a")
                nc.sync.dma_start(at[:], a2t_d[k * KP:(k + 1) * KP, :])
                for mm in range(2):
                    lhsT = xw2_sb[:, k, mm * KP:(mm + 1) * KP]
                    for ci, (c0, c1) in enumerate(ch2):
                        nc.tensor.matmul(ps2[mm][ci][:, :c1 - c0], lhsT,
                                         at[:, c0:c1],
                                         start=(k == 0), stop=(k == MT - 1))
            h2t_sb = big.tile([KP, 2, L], f16)
            for mm in range(2):
                for ci, (c0, c1) in enumerate(ch2):
                    nc.scalar.activation(h2t_sb[:, mm, c0:c1],
                                         ps2[mm][ci][:, :c1 - c0], AF.Relu,
                                         bias=b2c_sb[:, mm:mm + 1])

            # ---- GI = W_ih @ h2T + (b_ih [+ b_hh for r,z]) ----
            gi_sb = big.tile([KP, 6, L], f16)
            for c0, c1 in ch2:
                psg = [pspool.tile([KP, 512], f32, tag=f"g{m}", name=f"psgi_{m}") for m in range(6)]
                for m in range(6):
                    for k in range(2):
                        nc.tensor.matmul(psg[m][:, :c1 - c0],
                                         wiht_sb[:, k, m * KP:(m + 1) * KP],
                                         h2t_sb[:, k, c0:c1],
                                         start=(k == 0), stop=(k == 1))
                    nc.scalar.activation(gi_sb[:, m, c0:c1], psg[m][:, :c1 - c0],
                                         AF.Identity, bias=gib_sb[:, m:m + 1])
            # per-core GI patch on the first HALO columns (core 0 kills its pads)
            for m in range(6):
                nc.vector.tensor_scalar(gi_sb[:, m, :HALO], gi_sb[:, m, :HALO],
                                        patch_sb[:, m:m + 1],
                                        patch_sb[:, 6 + m:7 + m],
                                        ALU.mult, ALU.add)

            # ---- GRU fixed-point sweeps ----
            hsh_sb = big.tile([KP, 2, L + 1], f16)
            for mm in range(2):
                nc.vector.memset(hsh_sb[:, mm, :], 0.0)
            # Per sweep: gates from previous-sweep h (Jacobi), then the exact
            # affine scan.  The next sweep's 4 GI-identity matmuls are emitted
            # before the scans so the PE has work during the scan gap (keeps
            # the HAM clock warm).
            prefetched = None
            for s in range(SWEEPS):
                z_sb = big.tile([KP, 2, L], f32, tag="Z")
                b_sb = big.tile([KP, 2, L], f32, tag="B")
                for ci, (c0, c1) in enumerate(ch2):
                    cw = c1 - c0
                    if ci == 0 and prefetched is not None:
                        psg = prefetched
                        prefetched = None
                    else:
                        psg = [pspool.tile([KP, 512], f32, tag=f"g{m}",
                                           name=f"psu_{m}") for m in range(6)]
                        # u_rz = GI_rz (identity matmul) + W_hh_rz @ h_prev
                        for m in range(4):
                            nc.tensor.matmul(psg[m][:, :cw], ident_sb[:],
                                             gi_sb[:, m, c0:c1],
                                             start=True, stop=False)
                    for m in range(6):
                        for k in range(2):
                            nc.tensor.matmul(psg[m][:, :cw],
                                             whht_sb[:, k, m * KP:(m + 1) * KP],
                                             hsh_sb[:, k, c0:c1],
                                             start=(m >= 4 and k == 0),
                                             stop=(k == 1))
                    for mm in range(2):
                        r_t = tpool.tile([KP, 512], f16, tag="r")
                        ghn_t = tpool.tile([KP, 512], f16, tag="ghn")
                        t_t = tpool.tile([KP, 512], f16, tag="t")
                        un_t = tpool.tile([KP, 512], f16, tag="un")
                        n_t = tpool.tile([KP, 512], f32, tag="n")
                        t2_t = tpool.tile([KP, 512], f32, tag="t2")
                        nc.scalar.activation(r_t[:, :cw], psg[mm][:, :cw],
                                             AF.Sigmoid)
                        nc.scalar.activation(z_sb[:, mm, c0:c1],
                                             psg[2 + mm][:, :cw], AF.Sigmoid)
                        nc.scalar.activation(ghn_t[:, :cw], psg[4 + mm][:, :cw],
                                             AF.Identity,
                                             bias=bhn_sb[:, mm:mm + 1])
                        nc.vector.tensor_mul(t_t[:, :cw], r_t[:, :cw],
                                             ghn_t[:, :cw])
                        nc.vector.tensor_add(un_t[:, :cw], t_t[:, :cw],
                                             gi_sb[:, 4 + mm, c0:c1])
                        nc.scalar.activation(n_t[:, :cw], un_t[:, :cw], AF.Tanh)
                        # b = (1-z)*n = n - z*n  (keeps ScalarE off the path)
                        nc.vector.tensor_mul(t2_t[:, :cw], z_sb[:, mm, c0:c1],
                                             n_t[:, :cw])
                        nc.vector.tensor_sub(b_sb[:, mm, c0:c1], n_t[:, :cw],
                                             t2_t[:, :cw])
                if s < SWEEPS - 1:
                    # PE filler during the scans: next sweep's chunk-0 GI load
                    c0, c1 = ch2[0]
                    psg = [pspool.tile([KP, 512], f32, tag=f"g{m}",
                                       name=f"psp_{m}") for m in range(6)]
                    for m in range(4):
                        nc.tensor.matmul(psg[m][:, :c1 - c0], ident_sb[:],
                                         gi_sb[:, m, c0:c1],
                                         start=True, stop=False)
                    prefetched = psg
                # exact h recurrence: h_t = z_t * h_{t-1} + (1-z_t) n_t
                for mm in range(2):
                    nc.vector.tensor_tensor_scan(
                        hsh_sb[:, mm, 1:L + 1], z_sb[:, mm, :], b_sb[:, mm, :],
                        0.0, ALU.mult, ALU.add)

            # ---- final Linear on the real rows (skip halo) ----
            out_sb = cpool.tile([4, ROWS], f32)
            for c0, c1 in ch1:
                cw = c1 - c0
                psf = psxw.tile([KP, 512], f32, tag="xwps")
                for k in range(2):
                    nc.tensor.matmul(psf[:OUT, :cw], fcwt_sb[:, k, :],
                                     hsh_sb[:, k, HALO + 1 + c0:HALO + 1 + c1],
                                     start=(k == 0), stop=(k == 1))
                nc.scalar.activation(out_sb[:OUT, c0:c1], psf[:OUT, :cw],
                                     AF.Identity, bias=fcb_sb[:OUT, :])
            nc.sync.dma_start(out_d[:], out_sb[:OUT, :])

    nc.compile()
    return nc


def host_prepare(inputs):
    """Build the per-core input maps from the full problem inputs."""
    x = np.asarray(inputs["x"], np.float32)
    ei = np.asarray(inputs["edge_index"])
    W1 = np.asarray(inputs["W1"], np.float32)
    b1 = np.asarray(inputs["b1"], np.float32)
    W2 = np.asarray(inputs["W2"], np.float32)
    b2 = np.asarray(inputs["b2"], np.float32)
    W_ih = np.asarray(inputs["W_ih"], np.float32)
    W_hh = np.asarray(inputs["W_hh"], np.float32)
    b_ih = np.asarray(inputs["b_ih"], np.float32)
    b_hh = np.asarray(inputs["b_hh"], np.float32)
    fc_w = np.asarray(inputs["fc_w"], np.float32)
    fc_b = np.asarray(inputs["fc_b"], np.float32)

    N = NUM_NODES
    NPAD = MT * KP
    src, dst = ei[0].astype(np.int64), ei[1].astype(np.int64)
    deg = np.bincount(dst, minlength=N).astype(np.float64) + 1.0
    dinv = 1.0 / np.sqrt(deg)
    # A_T[s, d] = normalization weight of edge s->d (plus self loops)
    at = np.zeros((N, N), np.float32)
    np.add.at(at, (src, dst), (dinv[src] * dinv[dst]).astype(np.float32))
    idx = np.arange(N)
    at[idx, idx] += (dinv * dinv).astype(np.float32)
    at16 = at.astype(np.float16)
    del at

    atp = np.zeros((NPAD, L), np.float16)

    common = {
        "xt": np.concatenate(
            [x.T.astype(np.float16),
             np.zeros((IN_FEAT, NPAD - N), np.float16)], axis=1),
        "w1": W1.astype(np.float16),
        "w2": W2.astype(np.float16),
        "wiht": W_ih.T.astype(np.float16),
        "whht": W_hh.T.astype(np.float16),
        "fcwt": fc_w.T.astype(np.float16),
        "ident": np.eye(KP, dtype=np.float16),
        "b1c": b1.reshape(2, KP).T.astype(np.float32).copy(),
        "b2c": b2.reshape(2, KP).T.astype(np.float32).copy(),
        "gib": (b_ih + np.concatenate([b_hh[:2 * HID],
                                       np.zeros(HID, np.float32)])
                ).reshape(6, KP).T.astype(np.float32).copy(),
        "bhn": b_hh[2 * HID:].reshape(2, KP).T.astype(np.float32).copy(),
        "fcb": np.concatenate([fc_b, np.zeros(KP - OUT, np.float32)]
                              ).reshape(KP, 1),
    }

    in_maps = []
    for c in range(CORES):
        r0, r1 = c * ROWS, (c + 1) * ROWS
        a2t = atp.copy()
        if c == 0:
            a2t[:N, HALO:] = at16[:, r0:r1]
        else:
            a2t[:N, :] = at16[:, r0 - HALO:r1]
        patch = np.zeros((KP, 12), np.float32)
        if c == 0:
            # mul=0; add=-60 for r,z gate tiles, 0 for n tiles -> pad cols
            # produce exactly h=0 so row 0 starts from the true h0=0.
            patch[:, 6:10] = -60.0
        else:
            patch[:, 0:6] = 1.0
        in_maps.append({**common, "a2t": a2t, "patch": patch})
    return in_maps


def assemble_output(results):
    outs = [r["out_t"].T for r in results]          # each [ROWS, OUT]
    full = np.concatenate(outs, axis=0).astype(np.float32)
    return full[None]                               # [1, N, OUT]


def kernel(**inputs) -> np.ndarray:
    from concourse import bass_utils

    if "nc" not in _CACHE:
        _CACHE["nc"] = build_program()
    nc = _CACHE["nc"]
    in_maps = host_prepare(inputs)
    res = bass_utils.run_bass_kernel_spmd(
        nc, in_maps, core_ids=list(range(CORES)))
    return assemble_output(res.results)


if __name__ == "__main__":
    import reference

    inputs = {k: np.asarray(v) for k, v in reference.setup_inputs().items()}
    out = kernel(**inputs)
    print("kernel out", out.shape, out.dtype)
    np.save("/root/problem/kernel_out.npy", out)
